# revision 2
# baseline (speedup 1.0000x reference)
"""CRF decoder loss kernel for Trainium2 (8 NeuronCores, data-parallel batch).

Algorithm (mathematically identical to the reference): the reference loss is
mean_b(Zp - score) where Zp is the CRF log-partition of
log_softmax(enc @ W + b) and score is the gold-path score.  Writing
logits = R - logZ, the log-softmax normalizer cancels between Zp and score,
so the device never computes a softmax.  With a constant shift kappa the
forward recursion runs in LINEAR space:

    P_t = (P_{t-1} @ expT) * G_t,   G_t = exp((enc_t @ W + b - kappa))

and loss_b = log Zp_dev[b] - sum_{t<len_b}(R[t,b,tgt]-kappa) - gold-path
transition/start/end terms (host, f64, tiny).

Device structure per core (batch shard of 32, v-major state [256v x 32b]):
  - The 511-step recursion is split into two independent chains that
    interleave on the engines (hides the psum->sbuf round-trip latency):
      fwd: F_t for t in [0,255]; per step 4 fp8 matmuls (128x128 blocks of
           expT stationary) + one fused DVE multiply by G_t.
      bwd: suffix vectors c_u for u in [511,256] seeded by "injections"
           e*onehot(len_b-1==u) (a K=128 matmul whose one-hot rows are DATA),
           same 4-matmul + DVE-multiply step using expT^T blocks.
  - Zp for len<=256: paired extraction S_t = e . F_t every 2 steps
    (K=128/M=2/N=128 matmul into a [2,...] psum; host sums the 2 partials).
    Zp for len>256: single final dot F_255 . c_255.
  - Projection: fp8 DoubleRow matmuls (K=256, W*16 in fp8, ACT applies
    exp(psum/16 + b - kappa)), one matmul interleaved between scan steps,
    chunks consumed from both sequence ends (fwd ascending, bwd descending).
  - The program is compiled per distinct `lengths` (injection/extraction
    schedules are culled to the union over cores); one-hot rows stay data,
    so any lengths value is handled correctly.

Numerics: fp8e4 transition/projection with bf16 state; validated vs f64
reference: loss rel err ~7.5e-4 (tolerance 2e-2).
"""

import numpy as np
import ml_dtypes

import concourse.bacc as bacc
import concourse.tile as tile
from concourse import mybir
from concourse.bass_utils import run_bass_kernel_spmd

bf16 = ml_dtypes.bfloat16
fp8 = ml_dtypes.float8_e4m3
f32 = mybir.dt.float32
bf16_t = mybir.dt.bfloat16
fp8_t = mybir.dt.float8e4

S, B, H, V = 512, 256, 512, 256
NCORES = 8
BC = B // NCORES            # 32 batch per core
KAPPA = 6.05
CHUNK = 512                 # projection chunk rows = 16 steps * 32 batch
NCHUNK = 32
NSUP = 255                  # supersteps 1..255
RING = 8

_nc_cache = {}


def _build(inj_steps, ext_pairs):
    nc = bacc.Bacc("TRN2", debug=False)

    encT = nc.dram_tensor("encT", [128, NCHUNK, 4, CHUNK], fp8_t, kind="ExternalInput")
    wblk = nc.dram_tensor("wblk", [128, 2, 2, 2, 128], fp8_t, kind="ExternalInput")
    fwd8 = nc.dram_tensor("fwd8", [128, 4, 128], fp8_t, kind="ExternalInput")
    bwd8 = nc.dram_tensor("bwd8", [128, 4, 128], fp8_t, kind="ExternalInput")
    biasT = nc.dram_tensor("biasT", [128, 2], f32, kind="ExternalInput")
    estartT = nc.dram_tensor("estartT", [128, 2], f32, kind="ExternalInput")
    eendc2 = nc.dram_tensor("eendc2", [128, 2], bf16_t, kind="ExternalInput")
    e2pad = nc.dram_tensor("e2pad", [128, 128], fp8_t, kind="ExternalInput")
    ones2 = nc.dram_tensor("ones2", [128, 2], bf16_t, kind="ExternalInput")
    injrows = nc.dram_tensor("injrows", [128, 256 * 64], fp8_t, kind="ExternalInput")

    s_out = nc.dram_tensor("s_out", [2, 129, 128], f32, kind="ExternalOutput")

    with tile.TileContext(nc) as tc:
        with (
            tc.tile_pool(name="consts", bufs=1) as consts,
            tc.tile_pool(name="encp", bufs=4) as encp,
            tc.tile_pool(name="gpool", bufs=1) as gpool,
            tc.tile_pool(name="proj_ps", bufs=1, space="PSUM") as proj_ps,
            tc.tile_pool(name="fps", bufs=2, space="PSUM") as fps,
            tc.tile_pool(name="bps", bufs=2, space="PSUM") as bps,
            tc.tile_pool(name="sps", bufs=2, space="PSUM") as sps,
        ):
            w_sb = consts.tile([128, 2, 2, 2, 128], fp8_t)
            fwd8_sb = consts.tile([128, 4, 128], fp8_t)
            bwd8_sb = consts.tile([128, 4, 128], fp8_t)
            bias_sb = consts.tile([128, 2], f32)
            estart_sb = consts.tile([128, 2], f32)
            eendc_sb = consts.tile([128, 2], bf16_t)
            e2pad_sb = consts.tile([128, 128], fp8_t)
            ones_sb = consts.tile([128, 2], bf16_t)
            inj_sb = consts.tile([128, 256 * 64], fp8_t)
            s_sb = consts.tile([2, 129, 128], f32)
            fring = consts.tile([128, RING, 2, BC], bf16_t)
            bring = consts.tile([128, RING, 2, BC], bf16_t)
            ftmp = consts.tile([128, 2, BC], bf16_t)

            for dst, src in (
                (w_sb, wblk), (fwd8_sb, fwd8), (bwd8_sb, bwd8),
                (bias_sb, biasT), (estart_sb, estartT), (eendc_sb, eendc2),
                (e2pad_sb, e2pad), (ones_sb, ones2), (inj_sb, injrows),
            ):
                nc.sync.dma_start(out=dst[:], in_=src[:])

            gtiles = [gpool.tile([128, 16, 2, BC], bf16_t, name=f"g{c}", tag=f"g{c}")
                      for c in range(NCHUNK)]

            # ---------------- projection micro-op scheduler ----------------
            def emit_chunk(c, ops):
                """Append one closure per DoubleRow matmul (K=256 each); DMA
                rides on the first, ACT eviction (exp, scale=1/16 for the W*16
                fp8 trick) after the 2nd of each vh."""
                state = {}

                def dma():
                    et = encp.tile([128, 4, CHUNK], fp8_t, name="et", tag="enc")
                    nc.sync.dma_start(out=et[:], in_=encT[:, c, :, :])
                    state["et"] = et

                for vh in range(2):
                    for hp in range(2):
                        def op(vh=vh, hp=hp, first=(vh == 0 and hp == 0)):
                            if first:
                                dma()
                            if hp == 0:
                                state["ps"] = proj_ps.tile(
                                    [128, 16, BC], f32, name="pps", tag=f"pps{vh}")
                            nc.tensor.matmul(
                                state["ps"][:, :, :],
                                lhsT=w_sb[:, hp, :, vh, :],
                                rhs=state["et"][:, hp * 2:hp * 2 + 2, :],
                                start=(hp == 0),
                                stop=(hp == 1),
                                perf_mode=mybir.MatmulPerfMode.DoubleRow,
                            )
                            if hp == 1:
                                nc.scalar.activation(
                                    gtiles[c][:, :, vh, :], state["ps"][:, :, :],
                                    mybir.ActivationFunctionType.Exp,
                                    bias=bias_sb[:, vh:vh + 1], scale=1.0 / 16.0,
                                )
                        ops.append(op)

            # prologue chunks (both ends, lead of 2 blocks each side)
            pro_ops = []
            for c in (0, 31, 1, 30):
                emit_chunk(c, pro_ops)
            for op in pro_ops:
                op()

            # steady-state: block k emits pair (k+2, 29-k), one mm/superstep
            proj_ops = []
            for k in range(14):
                emit_chunk(k + 2, proj_ops)
                emit_chunk(29 - k, proj_ops)

            # ---------------- chain initialization ----------------
            # F_0 = g_0 * expStart  (fwd slot 0)
            for ih in range(2):
                nc.vector.tensor_scalar_mul(
                    fring[:, 0, ih, :],
                    in0=gtiles[0][:, 0, ih, :],
                    scalar1=estart_sb[:, ih:ih + 1],
                )
            # D_511 = g_511 * (e ⊗ m_511)   (bwd slot 0)
            sp_cur = [None]
            bp0 = bps.tile([128, 2, BC], f32, name="bp", tag="bp")
            nc.tensor.matmul(
                bp0[:, :, :], lhsT=e2pad_sb[:, :], rhs=inj_sb[:, 0:64],
                start=True, stop=True)
            nc.vector.tensor_tensor(
                out=bring[:, 0, :, :], in0=bp0[:, :, :],
                in1=gtiles[31][:, 15, :, :], op=mybir.AluOpType.mult)

            # ---------------- supersteps ----------------
            for s in range(1, NSUP + 1):
                slot, pslot = s % RING, (s - 1) % RING
                u = 511 - s
                gf = gtiles[s // 16]
                gb = gtiles[u // 16]
                blk, off = (s - 1) // 16, (s - 1) % 16
                # fwd chain first: psum_j = sum_i expT[i,j] F[i]
                fp = fps.tile([128, 2, BC], f32, name="fp", tag="fp")
                for jh in range(2):
                    for ih in range(2):
                        nc.tensor.matmul(
                            fp[:, jh, :],
                            lhsT=fwd8_sb[:, ih * 2 + jh, :],
                            rhs=fring[:, pslot, ih, :],
                            start=(ih == 0), stop=(ih == 1),
                        )
                nc.vector.tensor_tensor(
                    out=fring[:, slot, :, :], in0=gf[:, s % 16, :, :],
                    in1=fp[:, :, :],
                    op=mybir.AluOpType.mult)
                # fillers between the chains: proj op + injection matmul
                if off % 2 == 0 and blk < 14:
                    proj_ops[blk * 8 + off // 2]()
                bp = bps.tile([128, 2, BC], f32, name="bp", tag="bp")
                has_inj = s in inj_steps
                if has_inj:
                    nc.tensor.matmul(
                        bp[:, :, :], lhsT=e2pad_sb[:, :],
                        rhs=inj_sb[:, s * 64:(s + 1) * 64],
                        start=True, stop=False)
                # bwd chain: psum_i = sum_j expT[i,j] D[j]
                for ih in range(2):
                    for jh in range(2):
                        nc.tensor.matmul(
                            bp[:, ih, :],
                            lhsT=bwd8_sb[:, jh * 2 + ih, :],
                            rhs=bring[:, pslot, jh, :],
                            start=(jh == 0 and not has_inj),
                            stop=(ih == 1 and jh == 1),
                        )
                nc.vector.tensor_tensor(
                    out=bring[:, slot, :, :], in0=gb[:, u % 16, :, :],
                    in1=bp[:, :, :],
                    op=mybir.AluOpType.mult)
                # delayed paired extraction: at odd s>=3, pair p=(s-3)//2 from
                # ring slots (s-3)%4,(s-2)%4 (contiguous, written 2+ steps ago)
                if s % 2 == 1 and s >= 3 and ((s - 3) // 2) in ext_pairs:
                    p = (s - 3) // 2
                    if sp_cur[0] is None:
                        sp_cur[0] = sps.tile([2, 4, 128], f32, name="sp", tag="sp")
                    w0 = (s - 3) % RING
                    nc.tensor.matmul(
                        sp_cur[0][:, p % 4, :], lhsT=eendc_sb[:, :],
                        rhs=fring[:, w0:w0 + 2, :, :], start=True, stop=True)
                if s % 8 == 1 and s >= 9 and sp_cur[0] is not None:
                    nc.scalar.copy(
                        s_sb[:, ((s - 9) // 8) * 4:((s - 9) // 8) * 4 + 4, :],
                        sp_cur[0][:, :, :])
                    sp_cur[0] = None
            # ---------------- epilogue ----------------
            # trailing extractions (pairs extracted at s = 2p+3 > 255)
            for p in (127,):
                if p in ext_pairs:
                    if sp_cur[0] is None:
                        sp_cur[0] = sps.tile([2, 4, 128], f32, name="sp", tag="sp")
                    w0 = (2 * p) % RING
                    nc.tensor.matmul(
                        sp_cur[0][:, p % 4, :], lhsT=eendc_sb[:, :],
                        rhs=fring[:, w0:w0 + 2, :, :], start=True, stop=True)
            if sp_cur[0] is not None:
                nc.scalar.copy(s_sb[:, 124:128, :], sp_cur[0][:, :, :])
                sp_cur[0] = None
            # c_255 = M @ D_256  (raw, no g multiply)
            cp = bps.tile([128, 2, BC], f32, name="cp", tag="bp")
            lslot = NSUP % RING
            for ih in range(2):
                for jh in range(2):
                    nc.tensor.matmul(
                        cp[:, ih, :],
                        lhsT=bwd8_sb[:, jh * 2 + ih, :],
                        rhs=bring[:, lslot, jh, :],
                        start=(jh == 0), stop=(ih == 1 and jh == 1),
                    )
            nc.vector.tensor_tensor(
                out=ftmp[:, :, :], in0=cp[:, :, :],
                in1=fring[:, lslot, :, :], op=mybir.AluOpType.mult)
            spF = sps.tile([2, 4, 128], f32, name="sp", tag="sp")
            nc.tensor.matmul(
                spF[:, 0, 0:64], lhsT=ones_sb[:, :], rhs=ftmp[:, :, :],
                start=True, stop=True)
            nc.scalar.copy(s_sb[:, 128:129, :], spF[:, 0:1, :])

            nc.sync.dma_start(out=s_out[:], in_=s_sb[:])

    nc.compile()
    return nc


def _schedules(lens):
    inj_steps = set()
    ext_pairs = set()
    for b in range(B):
        L = int(lens[b])
        if L - 1 >= 256:
            inj_steps.add(511 - (L - 1))
        else:
            ext_pairs.add((L - 1) // 2)
    inj_steps.discard(0)  # s=0 handled by the always-emitted init matmul
    return inj_steps, ext_pairs


def _host_consts(d):
    W_ = np.asarray(d["W"], dtype=np.float32)
    b_ = np.asarray(d["b"], dtype=np.float64)
    T_ = np.asarray(d["transition"], dtype=np.float64)
    start_ = np.asarray(d["start_transition"], dtype=np.float64)
    end_ = np.asarray(d["end_transition"], dtype=np.float64)
    # W*16 in fp8 (values ~N(0, 0.7^2)); the 1/16 is folded into the ACT scale.
    # DR layout [h0, hp, kt, vh, v0] with h = hp*256 + kt*128 + h0.
    W16 = (W_ * 16.0).reshape(2, 2, 128, 2, 128)      # [hp, kt, h0, vh, v0]
    Wb = np.ascontiguousarray(W16.transpose(2, 0, 1, 3, 4)).astype(fp8)
    eT = np.exp(T_).astype(np.float32)
    fwd8b = np.ascontiguousarray(
        eT.reshape(2, 128, 2, 128).transpose(1, 0, 2, 3).reshape(128, 4, 128)
    ).astype(fp8)
    bwd8b = np.ascontiguousarray(
        eT.T.reshape(2, 128, 2, 128).transpose(1, 0, 2, 3).reshape(128, 4, 128)
    ).astype(fp8)
    biasT = np.ascontiguousarray((b_ - KAPPA).reshape(2, 128).T).astype(np.float32)
    estartT = np.ascontiguousarray(np.exp(start_).reshape(2, 128).T).astype(np.float32)
    ee = np.exp(end_)
    eendc2 = np.ascontiguousarray(ee.reshape(2, 128).T).astype(bf16)
    e2pad = np.zeros((128, 128), dtype=np.float32)
    e2pad[0, :] = ee[:128]
    e2pad[1, :] = ee[128:]
    e2pad = e2pad.astype(fp8)
    ones2 = np.ones((128, 2), dtype=bf16)
    return Wb, fwd8b, bwd8b, biasT, estartT, eendc2, e2pad, ones2


def _prep_core_inputs(core, enc_f8, lens, consts):
    Wb, fwd8b, bwd8b, biasT, estartT, eendc2, e2pad, ones2 = consts
    b0 = core * BC
    e = enc_f8[:, b0:b0 + BC, :].transpose(2, 0, 1).reshape(4, 128, NCHUNK, CHUNK)
    e = np.ascontiguousarray(e.transpose(1, 2, 0, 3))
    inj = np.zeros((128, 256, 64), dtype=fp8)
    for bl in range(BC):
        L = int(lens[b0 + bl])
        if L - 1 >= 256:
            s = 511 - (L - 1)
            inj[0, s, bl] = 1.0
            inj[1, s, 32 + bl] = 1.0
    return {
        "encT": e, "wblk": Wb, "fwd8": fwd8b, "bwd8": bwd8b, "biasT": biasT,
        "estartT": estartT, "eendc2": eendc2, "e2pad": e2pad, "ones2": ones2,
        "injrows": inj.reshape(128, 256 * 64),
    }


def kernel(enc_outs, W, b, transition, start_transition, end_transition,
           targets, lengths):
    enc = np.asarray(enc_outs, dtype=np.float32)
    W_ = np.asarray(W, dtype=np.float32)
    b_ = np.asarray(b, dtype=np.float64)
    T_ = np.asarray(transition, dtype=np.float64)
    start_ = np.asarray(start_transition, dtype=np.float64)
    end_ = np.asarray(end_transition, dtype=np.float64)
    tgt = np.asarray(targets).astype(np.int64)
    lens = np.asarray(lengths).astype(np.int64)

    inj_steps, ext_pairs = _schedules(lens)
    key = lens.tobytes()
    if key not in _nc_cache:
        _nc_cache[key] = _build(inj_steps, ext_pairs)
    nc = _nc_cache[key]

    consts = _host_consts({
        "W": W, "b": b, "transition": transition,
        "start_transition": start_transition, "end_transition": end_transition,
    })
    enc_f8 = enc.astype(fp8)
    in_maps = [_prep_core_inputs(c, enc_f8, lens, consts) for c in range(NCORES)]
    res = run_bass_kernel_spmd(nc, in_maps, list(range(NCORES))).results

    # ---------------- host epilogue (small inputs only) ----------------
    tmask = (np.arange(S)[:, None] < lens[None, :])
    trans_sum = (T_[tgt[:-1], tgt[1:]] * tmask[1:]).sum(axis=0)
    last_tgt = tgt[lens - 1, np.arange(B)]
    hostscore = start_[tgt[0]] + trans_sum + end_[last_tgt]

    Wg = W_.T[tgt.reshape(-1)]
    emis_all = (np.einsum("rh,rh->r", enc.reshape(S * B, H), Wg,
                          optimize=True).reshape(S, B)
                + b_[tgt])
    emis = ((emis_all - KAPPA) * tmask).sum(axis=0)

    loss_b = np.zeros(B, dtype=np.float64)
    for c in range(NCORES):
        b0 = c * BC
        r = np.asarray(res[c]["s_out"], dtype=np.float64)  # [2, 129, 128]
        rp = r[:, :128, :].reshape(2, 128, 2, 2, 32)       # [row, pair, q, ih, b]
        Sfull = (rp[0, :, :, 0, :] + rp[1, :, :, 1, :]).reshape(256, 32)
        fin = r[0, 128, :32] + r[1, 128, 32:64]            # (32,) bwd Zp
        bl = lens[b0:b0 + BC]
        blocal = np.arange(BC)
        zp = np.where(bl <= 256, Sfull[np.minimum(bl - 1, 255), blocal], fin)
        loss_b[b0:b0 + BC] = np.log(zp) - emis[b0:b0 + BC] - hostscore[b0:b0 + BC]

    return np.float32(loss_b.mean())
